# revision 35
# baseline (speedup 1.0000x reference)
"""Trainium2 Bass kernel for the batched MPS quantum-circuit forward pass.

Polar-form closed solution. Each gate is U_l = RZ(z)RY(y); the chained site
tensor entry is a product of 4 gate entries g_l(row_l, col_l) with
row_l = a_l ^ m_l, col_l = a_{l-1}.  Writing each entry as
(+-cos/sin(y_l/2)) * e^{+-i z_l/2} (phase sign = row bit), every output
element is  Mg * e^{i phi}  where Mg (signed magnitude) is a product of 4
real table entries and phi = sum_l (2*row_l - 1) * z_l/2 depends only on the
row bits t = (t0t1t2t3).

In t-coordinates (t_l = a_l ^ m_l) the value is independent of m3 and of the
rho-placement, so an interior site has only 8(m0m1m2) x 16(t) = 128 unique
complex values; the XOR placement rho = t ^ lam is a static permutation done
on the host.  Per site the device computes:
    Mg[m012, t] = M01x[m0, t0, t1] * M23[m1m2, t1t2t3]    (128 muls)
    im = Mg * sinT[t],  re = Mg * cosT[t]                 (2 x 128 muls)
with sinT/cosT built from a range-reduced phase table (phi/2pi - round, via
the f16 magic-constant trick) and one Sin activation each.

Output is compact f16: 18 interior sites x 256 (im|re planes) + site0 (16
complex, lam=0 only) + site19 (16 complex: sum over t0t1t2, index (m012,t3)).
The host expands to the full (1024,20,16,16,2) complex64 tensor (structural
zeros + XOR reindex); no host arithmetic beyond re+i*im.

Sharding: pure data parallelism, batch 1024 = 8 cores x 128 partitions.
All tensor instructions use <=3 free dims (hw TENSOR3D limit).
"""

import sys

sys.path.insert(0, "/opt/trn_rl_repo")

import numpy as np

B_TOTAL = 1024
N_CORES = 8
B = B_TOTAL // N_CORES
NQ = 20
P_COLS = 160
OUT_COLS = 4672  # 3*1536 interior + 32 site0 + 32 site19

_CACHE = {}


def _build_nc():
    import concourse.bass as bass
    import concourse.tile as tile
    from concourse import bacc, mybir

    f32 = mybir.dt.float32
    f16 = mybir.dt.float16
    ALU = mybir.AluOpType
    MUL = ALU.mult
    SIN = mybir.ActivationFunctionType.Sin
    ABS = mybir.ActivationFunctionType.Abs

    TWO_PI = float(2 * np.pi)
    INV_4PI = float(1.0 / (4 * np.pi))
    # Round-to-int magic for the f32 ALU datapath (storage dtype is f16 but
    # the DVE computes in f32, so the f32 magic is the one that rounds).
    MAGIC = 12582912.0  # 1.5 * 2^23

    nc = bacc.Bacc("TRN2", target_bir_lowering=False, debug=False)
    theta_d = nc.dram_tensor("theta", [B, P_COLS], f32, kind="ExternalInput").ap()
    out_d = nc.dram_tensor("out", [B, OUT_COLS], f16, kind="ExternalOutput").ap()

    from contextlib import ExitStack

    with tile.TileContext(nc) as tc, ExitStack() as ctx:
        pool = ctx.enter_context(tc.tile_pool(name="main", bufs=1))

        def tl(name, w, dt=f16):
            return pool.tile([B, w], dt, name=name)

        th = tl("th", 160, f32)
        zs = tl("zs", 80)           # z/(4pi) f16 (l*20+q)
        pq = tl("pq", 160)          # q*8 + [P(t0t1):0..3 | Q(t2t3):4..7]
        phs = tl("phs", 320)        # q*16 + ts, ts = t0*8+t1*4+t2*2+t3
        kk = tl("kk", 320)          # round(phs)
        ww = tl("ww", 320)          # phs - round(phs) in [-.5,.5]
        nw = tl("nw", 320)
        aw = tl("aw", 320)
        sinT = tl("sinT", 320)      # sin(phi)  (plane 0 = im)
        cosT = tl("cosT", 320)      # cos(phi)
        gt = tl("gt", 320)          # (l*4 + row*2 + col)*20 + q
        m01x = tl("m01x", 640)      # q*32 + m0*16 + t0*8 + t1*4 + t2t3(dup)
        m23 = tl("m23", 640)        # q*32 + (m1*2+m2)*8 + t1*4 + t2*2 + t3
        mg = tl("mg", 2560)         # q*128 + m012*16 + ts
        outt = tl("outt", OUT_COLS)  # g*1536 + plane*768 + qg*128 + m012*16 + ts
        p19s = tl("p19s", 128)
        p19c = tl("p19c", 128)
        r1s = tl("r1s", 64)
        r2s = tl("r2s", 32)
        r1c = tl("r1c", 64)
        r2c = tl("r2c", 32)
        halfpi = tl("halfpi", 1, f32)
        warm = tl("warm", 1, f32)

        def ap(t, off, dims):
            w = t.shape[1]
            return bass.AP(tensor=t.tensor, offset=t.offset + off, ap=[[w, B]] + dims)

        # ---- warm the Sin activation table before theta arrives ----------
        nc.vector.memset(halfpi[:], float(np.pi / 2))
        nc.scalar.activation(warm[:], halfpi[:], SIN, scale=0.5)
        nc.sync.dma_start(th[:], theta_d)

        # ---- Act: gate tables -------------------------------------------
        # cos(y/2) = Sin(pi/2 - y/2) directly; for the rare y < -pi the Sin
        # table arg exceeds pi where its error is still only ~1e-3 (measured).
        yl = [[40, 4], [1, 20]]
        nc.scalar.activation(
            ap(gt, 0, [[80, 4], [60, 2], [1, 20]]),
            ap(th, 0, [[40, 4], [0, 2], [1, 20]]),
            SIN, bias=halfpi[:], scale=-0.5,
        )
        nc.scalar.activation(ap(gt, 40, [[80, 4], [1, 20]]), ap(th, 0, yl), SIN, scale=0.5)
        nc.scalar.activation(ap(gt, 20, [[80, 4], [1, 20]]), ap(th, 0, yl), SIN, scale=-0.5)

        # ---- DVE: phase chain -------------------------------------------
        nc.vector.tensor_scalar_mul(
            ap(zs, 0, [[20, 4], [1, 20]]), ap(th, 20, yl), INV_4PI
        )
        zin = lambda o: ap(zs, o, [[1, 20], [40, 2]])
        nc.vector.tensor_add(ap(pq, 3, [[8, 20], [4, 2]]), zin(0), zin(20))
        nc.vector.tensor_sub(ap(pq, 2, [[8, 20], [4, 2]]), zin(0), zin(20))
        nc.vector.tensor_scalar_mul(
            ap(pq, 0, [[8, 20], [4, 2], [1, 2]]),
            ap(pq, 3, [[8, 20], [4, 2], [-1, 2]]),
            -1.0,
        )
        # phs[q, ts] = P[t0t1] + Q[t2t3]; per-t0 instr, iter [t1, t2t3, q]
        for t0 in (0, 1):
            nc.vector.tensor_add(
                ap(phs, t0 * 8, [[4, 2], [1, 4], [16, 20]]),
                ap(pq, t0 * 2, [[1, 2], [0, 4], [8, 20]]),
                ap(pq, 4, [[0, 2], [1, 4], [8, 20]]),
            )
        nc.vector.tensor_scalar(kk[:], phs[:], MAGIC, MAGIC, ALU.add, ALU.subtract)
        nc.vector.tensor_sub(ww[:], phs[:], kk[:])

        # ---- Act: phase trig (abs on Act: it is idle waiting for ww) -----
        nc.scalar.activation(aw[:], ww[:], ABS)
        nc.scalar.activation(sinT[:], ww[:], SIN, scale=TWO_PI)
        nc.scalar.activation(cosT[:], aw[:], SIN, bias=halfpi[:], scale=-TWO_PI)

        # ---- m01x on DVE (early: Pool's mgBC half depends on it) ---------
        # m01x[q, m0, t0, t1, dup t2t3] = g0(t0,0)*g1(t1, t0^m0); iter [q,t1,t2t3]
        with tc.high_priority():
            for m0 in (0, 1):
                for t0 in (0, 1):
                    nc.vector.tensor_tensor(
                        ap(m01x, m0 * 16 + t0 * 8, [[32, 20], [4, 2], [1, 4]]),
                        ap(gt, t0 * 40, [[1, 20], [0, 2], [0, 4]]),
                        ap(gt, 80 + ((t0 ^ m0) * 20), [[1, 20], [40, 2], [0, 4]]),
                        MUL,
                    )

        # ---- m23 on Pool (m1=0) + DVE (m1=1) -----------------------------
        # m23[q,m12,t1t2t3] = g2(t2, t1^m1) * g3(t3, t2^m2); iter [t2,t3,q]
        def m23_build(eng, m1, m2, t1):
            eng.tensor_tensor(
                ap(m23, (m1 * 2 + m2) * 8 + t1 * 4, [[2, 2], [1, 2], [32, 20]]),
                ap(gt, 160 + ((t1 ^ m1) * 20), [[40, 2], [0, 2], [1, 20]]),
                ap(gt, 240 + m2 * 20, [[20 - m2 * 40, 2], [40, 2], [1, 20]]),
                MUL,
            )

        for m1 in (0, 1):
            for m2 in (0, 1):
                for t1 in (0, 1):
                    eng = nc.vector if (m1, m2) == (1, 1) else nc.gpsimd
                    m23_build(eng, m1, m2, t1)

        # ---- mg: per (m0,t0), iter [q, m12, t123] ------------------------
        def mg_mul(q0, nq, m0, t0, eng=None):
            (eng or nc.vector).tensor_tensor(
                ap(mg, q0 * 128 + m0 * 64 + t0 * 8, [[128, nq], [16, 4], [1, 8]]),
                ap(m01x, q0 * 32 + m0 * 16 + t0 * 8, [[32, nq], [0, 4], [1, 8]]),
                ap(m23, q0 * 32, [[32, nq], [8, 4], [1, 8]]),
                MUL,
            )

        def finals(base, q0, nq, plane, T):
            nc.vector.tensor_tensor(
                ap(outt, base + plane * nq * 128, [[128, nq], [16, 8], [1, 16]]),
                ap(mg, q0 * 128, [[128, nq], [16, 8], [1, 16]]),
                ap(T, q0 * 16, [[16, nq], [0, 8], [1, 16]]),
                MUL,
            )

        # group A: q1..6 (site0 from q0 block)
        mg_mul(0, 7, 0, 0)
        mg_mul(0, 7, 0, 1)
        mg_mul(1, 6, 1, 0)
        mg_mul(1, 6, 1, 1)
        with tc.high_priority():
            finals(0, 1, 6, 0, sinT)
            nc.sync.dma_start(out_d[:, 0:768], outt[:, 0:768])
            finals(0, 1, 6, 1, cosT)
            nc.sync.dma_start(out_d[:, 768:1536], outt[:, 768:1536])

        # groups B+C: q7..19; m0=1 half on Pool (its idle window)
        mg_mul(7, 13, 1, 0, nc.gpsimd)
        mg_mul(7, 13, 1, 1, nc.gpsimd)
        mg_mul(7, 13, 0, 0)
        mg_mul(7, 13, 0, 1)

        # site 0 on Pool: outt[4608 + plane*16 + ts] = mg[q0, m012=0] * T[q0]
        nc.gpsimd.tensor_tensor(
            ap(outt, 4608, [[1, 16]]), ap(mg, 0, [[1, 16]]), ap(sinT, 0, [[1, 16]]), MUL
        )
        nc.gpsimd.tensor_tensor(
            ap(outt, 4624, [[1, 16]]), ap(mg, 0, [[1, 16]]), ap(cosT, 0, [[1, 16]]), MUL
        )
        # site 19 (Pool, early so the merged C-re DMA is not tail-blocked)
        nc.gpsimd.tensor_tensor(
            p19s[:], ap(mg, 19 * 128, [[16, 8], [1, 16]]),
            ap(sinT, 19 * 16, [[0, 8], [1, 16]]), MUL,
        )
        nc.gpsimd.tensor_tensor(
            p19c[:], ap(mg, 19 * 128, [[16, 8], [1, 16]]),
            ap(cosT, 19 * 16, [[0, 8], [1, 16]]), MUL,
        )
        # reduce t0 (stride 8), then t1 (stride 4), then t2 (stride 2), keep t3
        for p, r1, r2, oc in ((p19s, r1s, r2s, 4640), (p19c, r1c, r2c, 4656)):
            nc.gpsimd.tensor_add(
                ap(r1, 0, [[8, 8], [1, 8]]),
                ap(p, 0, [[16, 8], [1, 8]]), ap(p, 8, [[16, 8], [1, 8]]),
            )
            nc.gpsimd.tensor_add(
                ap(r2, 0, [[4, 8], [1, 4]]),
                ap(r1, 0, [[8, 8], [1, 4]]), ap(r1, 4, [[8, 8], [1, 4]]),
            )
            nc.gpsimd.tensor_add(
                ap(outt, oc, [[2, 8], [1, 2]]),
                ap(r2, 0, [[4, 8], [1, 2]]), ap(r2, 2, [[4, 8], [1, 2]]),
            )
        with tc.high_priority():
            finals(1536, 7, 8, 0, sinT)
            nc.sync.dma_start(out_d[:, 1536:2560], outt[:, 1536:2560])
            finals(1536, 7, 8, 1, cosT)
            nc.sync.dma_start(out_d[:, 2560:3584], outt[:, 2560:3584])
            finals(3584, 15, 4, 0, sinT)
            finals(3584, 15, 4, 1, cosT)
            # whole group C + site0 + site19 in one DMA
            nc.sync.dma_start(out_d[:, 3584:4672], outt[:, 3584:4672])

    nc.compile()
    return nc


def _get_nc():
    if "nc" not in _CACHE:
        _CACHE["nc"] = _build_nc()
    return _CACHE["nc"]


# host-side static index maps (natural ts order: ts == t)
_LAM = np.arange(16)
_S = (_LAM[:, None] >> 1) * 16 + (_LAM[:, None] ^ _LAM[None, :])
_S19 = (_LAM[:, None] >> 1) * 2 + (np.arange(2)[None, :] ^ (_LAM[:, None] & 1))


def kernel(theta, batch_size):
    from concourse.bass_utils import run_bass_kernel_spmd

    theta = np.ascontiguousarray(np.asarray(theta), dtype=np.float32)
    assert theta.shape == (B_TOTAL, P_COLS)
    nc = _get_nc()
    in_maps = [{"theta": theta[c * B : (c + 1) * B]} for c in range(N_CORES)]
    res = run_bass_kernel_spmd(nc, in_maps, core_ids=list(range(N_CORES)))
    _CACHE["last_res"] = res
    buf = np.concatenate([r["out"] for r in res.results], axis=0).astype(np.float32)

    full = np.zeros((B_TOTAL, NQ, 16, 16, 2), np.complex64)
    vs = []
    for base, nq in ((0, 6), (1536, 8), (3584, 4)):
        g = buf[:, base : base + nq * 256].reshape(B_TOTAL, 2, nq, 128)
        vs.append(g[:, 1] + 1j * g[:, 0])
    v = np.concatenate(vs, axis=1)  # [b, 18, m012*16+ts]
    fi = v[:, :, _S]  # [b, 18, lam, rho]
    full[:, 1:19, :, 0::2, 0] = fi[..., 0::2]
    full[:, 1:19, :, 1::2, 1] = fi[..., 1::2]
    s0 = buf[:, 4608:4640].reshape(B_TOTAL, 2, 16)
    v0 = s0[:, 1] + 1j * s0[:, 0]
    full[:, 0, 0, 0::2, 0] = v0[:, 0::2]
    full[:, 0, 0, 1::2, 1] = v0[:, 1::2]
    s19 = buf[:, 4640:4672].reshape(B_TOTAL, 2, 16)
    v19 = s19[:, 1] + 1j * s19[:, 0]  # [b, m012*2+t3]
    full[:, 19, :, 0, 0] = v19[:, _S19[:, 0]]
    full[:, 19, :, 0, 1] = v19[:, _S19[:, 1]]
    return full
